# revision 23
# baseline (speedup 1.0000x reference)
"""ConnectionProductBlock on 8 TRN2 NeuronCores.

out[b, c*K + k, h, w] = am_out[b, c, h, w] * first_out[b, k, h, w]
  with B=16, C=8, K=64, H=W=56.

Strategy (data parallel over batch, 2 batches per core, no communication):
  - SBUF layout puts channels on partitions, hw (=3136) on the free dim so
    every DMA moves long contiguous runs (12.5KB per partition).
  - first_out for the core's 2 batches loads once as [128, 3136]
    (partition = b*64 + k).
  - am needs a partition-broadcast (am[b, c] replicated across the 64 k
    partitions of batch b). Compute engines have fixed lane<->partition
    wiring, so the replication is done on the idle TensorEngine: a K=2
    selector matmul sel.T @ am[{b0,b1}, c] writes rep[p, f] = am[p//64, c, f]
    into PSUM in 448-column chunks.
  - VectorEngine multiplies first * rep into an SBUF staging tile per c,
    which is DMAed out as one 1.6MB transfer.
HBM traffic per core is the 14.6MB minimum -> memory-roofline bound.
"""

import numpy as np

B, C, K, H, W = 16, 8, 64, 56, 56
HW = H * W  # 3136
NCORES = 8
BPC = B // NCORES  # batches per core = 2
CHUNK = 448  # 3136 = 7 * 448; one PSUM bank holds 448 fp32 comfortably
NCHUNK = HW // CHUNK

_PROGRAMS = {}


def _build_program(repeat=1):
    """repeat>1 wraps the whole body in a hardware loop; used only by the
    local benchmark harness to amortize dispatch overhead when timing."""
    import contextlib

    import concourse.bacc as bacc
    import concourse.mybir as mybir
    import concourse.tile as tile

    nc = bacc.Bacc("TRN2", debug=False)
    # am rows (partition = b*8 + c) with the selector blocks appended on the
    # free dim: one DMA covers both, so each matmul carries a single sem wait
    # (the Matmult instruction struct only has one sync-wait slot).
    amsel = nc.dram_tensor(
        "amsel", [BPC * C, HW + C * BPC * K], mybir.dt.float32, kind="ExternalInput"
    )
    first = nc.dram_tensor(
        "first", [BPC, K, HW], mybir.dt.float32, kind="ExternalInput"
    )
    out = nc.dram_tensor(
        "out", [BPC, C * K, HW], mybir.dt.float32, kind="ExternalOutput"
    )

    with tile.TileContext(nc) as tc:
        with (
            tc.tile_pool(name="ins", bufs=1) as ins_pool,
            tc.tile_pool(name="rep", bufs=8, space="PSUM") as psum_pool,
            tc.tile_pool(name="outs", bufs=3) as out_pool,
            tc.For_i(0, repeat, 1) if repeat > 1 else contextlib.nullcontext(),
        ):
            # first2[p] = first[p // 64, p % 64]  (both batches stacked)
            first2 = ins_pool.tile([BPC * K, HW], mybir.dt.float32)
            nc.sync.dma_start(
                out=first2[:], in_=first.ap().rearrange("b k f -> (b k) f")
            )
            # am2t[p, :HW] = am[p // 8, p % 8]  (partition = b*8 + c);
            # am2t[:, HW + c*128 : HW + (c+1)*128] = selector block for c.
            # With rhs = am2t[:, hw-chunk], sel_c.T @ rhs gives
            # rep[p, f] = am[p // 64, c, f]: block-broadcast of channel c of
            # each batch across that batch's 64 k-partitions. (PE requires rhs
            # base partition in {0, 32, 64}, so the selector — not a strided
            # rhs view — encodes the channel pick.)
            am2t = ins_pool.tile([BPC * C, HW + C * BPC * K], mybir.dt.float32)
            nc.sync.dma_start(out=am2t[:], in_=amsel.ap())

            # [c, b, k, f] view; out_r[c] ([2, 64, HW]) iterates (b, k, f) —
            # the same element order as a [128, HW] SBUF tile with
            # partition = b*64 + k, so one full-width DMA per c works.
            out_r = out.ap().rearrange("b (c k) f -> c b k f", c=C)
            for c in range(C):
                out_t = out_pool.tile([BPC * K, HW], mybir.dt.float32, tag="out")
                for j in range(NCHUNK):
                    f0 = j * CHUNK
                    rep = psum_pool.tile([BPC * K, CHUNK], mybir.dt.float32, tag="rep")
                    nc.tensor.matmul(
                        rep[:],
                        lhsT=am2t[
                            :, HW + c * BPC * K : HW + (c + 1) * BPC * K
                        ],
                        rhs=am2t[:, f0 : f0 + CHUNK],
                        start=True,
                        stop=True,
                    )
                    nc.vector.tensor_mul(
                        out_t[:, f0 : f0 + CHUNK],
                        first2[:, f0 : f0 + CHUNK],
                        rep[:],
                    )
                nc.sync.dma_start(out=out_r[c], in_=out_t[:])
    nc.compile()
    return nc


def _get_program(repeat=1):
    if repeat not in _PROGRAMS:
        _PROGRAMS[repeat] = _build_program(repeat)
    return _PROGRAMS[repeat]


def _make_sel():
    # sel[b*C + c, c*128 + b*64 + k] = 1  (lhsT layout: [K=16, M=128] per c)
    sel = np.zeros((BPC * C, C * BPC * K), dtype=np.float32)
    for c in range(C):
        for b in range(BPC):
            sel[b * C + c, c * BPC * K + b * K : c * BPC * K + (b + 1) * K] = 1.0
    return sel


def _run(am_np, first_np, **spmd_kwargs):
    from concourse.bass_utils import run_bass_kernel_spmd

    nc = _get_program()
    sel = _make_sel()
    in_maps = []
    for i in range(NCORES):
        am_i = am_np[BPC * i : BPC * (i + 1)].reshape(BPC * C, HW)
        amsel = np.concatenate([am_i, sel], axis=1)
        in_maps.append(
            {
                "amsel": np.ascontiguousarray(amsel),
                "first": np.ascontiguousarray(first_np[BPC * i : BPC * (i + 1)]),
            }
        )
    return run_bass_kernel_spmd(nc, in_maps, core_ids=list(range(NCORES)), **spmd_kwargs)


def kernel(am_out, first_out):
    am_np = np.asarray(am_out, dtype=np.float32).reshape(B, C, HW)
    first_np = np.asarray(first_out, dtype=np.float32).reshape(B, K, HW)
    res = _run(am_np, first_np)
    out = np.concatenate([res.results[i]["out"] for i in range(NCORES)], axis=0)
    return out.reshape(B, C * K, H, W)


# revision 25
# speedup vs baseline: 2.4575x; 2.4575x over previous
"""ConnectionProductBlock on 8 TRN2 NeuronCores.

out[b, c*K + k, h, w] = am_out[b, c, h, w] * first_out[b, k, h, w]
  with B=16, C=8, K=64, H=W=56.

Strategy (data parallel over batch, 2 batches per core, no communication):
  - SBUF layout puts channels on partitions, hw (=3136) on the free dim so
    every DMA moves long contiguous runs (12.5KB per partition).
  - first_out for the core's 2 batches loads once as [128, 3136]
    (partition = b*64 + k).
  - am needs a partition-broadcast (am[b, c] replicated across the 64 k
    partitions of batch b). Compute engines have fixed lane<->partition
    wiring, so the replication is done on the idle TensorEngine: a K=2
    selector matmul sel.T @ am[{b0,b1}, c] writes rep[p, f] = am[p//64, c, f]
    into PSUM in 448-column chunks.
  - VectorEngine multiplies first * rep into an SBUF staging tile per c,
    which is DMAed out as one 1.6MB transfer.
HBM traffic per core is the 14.6MB minimum -> memory-roofline bound.
"""

import numpy as np

B, C, K, H, W = 16, 8, 64, 56, 56
HW = H * W  # 3136
NCORES = 8
BPC = B // NCORES  # batches per core = 2
CHUNK = 448  # 3136 = 7 * 448; one PSUM bank holds 448 fp32 comfortably
NCHUNK = HW // CHUNK

_PROGRAMS = {}


def _build_program(repeat=1):
    """repeat>1 wraps the whole body in a hardware loop; used only by the
    local benchmark harness to amortize dispatch overhead when timing."""
    import contextlib

    import concourse.bacc as bacc
    import concourse.mybir as mybir
    import concourse.tile as tile

    nc = bacc.Bacc("TRN2", debug=False)
    # am rows (partition = b*8 + c) with the selector blocks appended on the
    # free dim: one DMA covers both, so each matmul carries a single sem wait
    # (the Matmult instruction struct only has one sync-wait slot).
    amsel = nc.dram_tensor(
        "amsel", [BPC * C, HW + C * BPC * K], mybir.dt.float32, kind="ExternalInput"
    )
    first = nc.dram_tensor(
        "first", [BPC, K, HW], mybir.dt.float32, kind="ExternalInput"
    )
    out = nc.dram_tensor(
        "out", [BPC, C * K, HW], mybir.dt.float32, kind="ExternalOutput"
    )

    with tile.TileContext(nc) as tc:
        with (
            tc.tile_pool(name="ins", bufs=1) as ins_pool,
            tc.tile_pool(name="rep", bufs=8, space="PSUM") as psum_pool,
            tc.tile_pool(name="outs", bufs=3) as out_pool,
            tc.For_i(0, repeat, 1) if repeat > 1 else contextlib.nullcontext(),
        ):
            # first2[p] = first[p // 64, p % 64]  (both batches stacked)
            first2 = ins_pool.tile([BPC * K, HW], mybir.dt.float32)
            nc.sync.dma_start(
                out=first2[:], in_=first.ap().rearrange("b k f -> (b k) f")
            )
            # am2t[p, :HW] = am[p // 8, p % 8]  (partition = b*8 + c);
            # am2t[:, HW + c*128 : HW + (c+1)*128] = selector block for c.
            # With rhs = am2t[:, hw-chunk], sel_c.T @ rhs gives
            # rep[p, f] = am[p // 64, c, f]: block-broadcast of channel c of
            # each batch across that batch's 64 k-partitions. (PE requires rhs
            # base partition in {0, 32, 64}, so the selector — not a strided
            # rhs view — encodes the channel pick.)
            am2t = ins_pool.tile([BPC * C, HW + C * BPC * K], mybir.dt.float32)
            nc.sync.dma_start(out=am2t[:], in_=amsel.ap())

            out_ap = out.ap()
            for c in range(C):
                out_t = out_pool.tile([BPC * K, HW], mybir.dt.float32, tag="out")
                for j in range(NCHUNK):
                    f0 = j * CHUNK
                    rep = psum_pool.tile([BPC * K, CHUNK], mybir.dt.float32, tag="rep")
                    nc.tensor.matmul(
                        rep[:],
                        lhsT=am2t[
                            :, HW + c * BPC * K : HW + (c + 1) * BPC * K
                        ],
                        rhs=am2t[:, f0 : f0 + CHUNK],
                        start=True,
                        stop=True,
                    )
                    nc.vector.tensor_mul(
                        out_t[:, f0 : f0 + CHUNK],
                        first2[:, f0 : f0 + CHUNK],
                        rep[:],
                    )
                # One DMA per batch ([64, HW] each, contiguous in DRAM).
                # b=0 goes on the SP HWDGE ring, b=1 on the ACT ring: the two
                # rings run concurrently, so both partition halves are in
                # flight and all 16 SBUF ports stay busy.
                for b, eng in ((0, nc.sync), (1, nc.scalar)):
                    eng.dma_start(
                        out=out_ap[b, c * K : (c + 1) * K, :],
                        in_=out_t[b * K : (b + 1) * K, :],
                    )
    nc.compile()
    return nc


def _get_program(repeat=1):
    if repeat not in _PROGRAMS:
        _PROGRAMS[repeat] = _build_program(repeat)
    return _PROGRAMS[repeat]


def _make_sel():
    # sel[b*C + c, c*128 + b*64 + k] = 1  (lhsT layout: [K=16, M=128] per c)
    sel = np.zeros((BPC * C, C * BPC * K), dtype=np.float32)
    for c in range(C):
        for b in range(BPC):
            sel[b * C + c, c * BPC * K + b * K : c * BPC * K + (b + 1) * K] = 1.0
    return sel


def _run(am_np, first_np, **spmd_kwargs):
    from concourse.bass_utils import run_bass_kernel_spmd

    nc = _get_program()
    sel = _make_sel()
    in_maps = []
    for i in range(NCORES):
        am_i = am_np[BPC * i : BPC * (i + 1)].reshape(BPC * C, HW)
        amsel = np.concatenate([am_i, sel], axis=1)
        in_maps.append(
            {
                "amsel": np.ascontiguousarray(amsel),
                "first": np.ascontiguousarray(first_np[BPC * i : BPC * (i + 1)]),
            }
        )
    return run_bass_kernel_spmd(nc, in_maps, core_ids=list(range(NCORES)), **spmd_kwargs)


def kernel(am_out, first_out):
    am_np = np.asarray(am_out, dtype=np.float32).reshape(B, C, HW)
    first_np = np.asarray(first_out, dtype=np.float32).reshape(B, K, HW)
    res = _run(am_np, first_np)
    out = np.concatenate([res.results[i]["out"] for i in range(NCORES)], axis=0)
    return out.reshape(B, C * K, H, W)


# revision 29
# speedup vs baseline: 2.6462x; 1.0768x over previous
"""ConnectionProductBlock on 8 TRN2 NeuronCores.

out[b, c*K + k, h, w] = am_out[b, c, h, w] * first_out[b, k, h, w]
  with B=16, C=8, K=64, H=W=56.

Strategy (data parallel over batch, 2 batches per core, no communication):
  - SBUF layout puts channels on partitions, hw (=3136) on the free dim so
    every DMA moves long contiguous runs (12.5KB per partition).
  - first_out for the core's 2 batches loads once as [128, 3136]
    (partition = b*64 + k).
  - am needs a partition-broadcast (am[b, c] replicated across the 64 k
    partitions of batch b). Compute engines have fixed lane<->partition
    wiring, so the replication is done on the idle TensorEngine: a K=2
    selector matmul sel.T @ am[{b0,b1}, c] writes rep[p, f] = am[p//64, c, f]
    into PSUM in 448-column chunks.
  - VectorEngine multiplies first * rep into an SBUF staging tile per c,
    which is DMAed out as one 1.6MB transfer.
HBM traffic per core is the 14.6MB minimum -> memory-roofline bound.
"""

import numpy as np

B, C, K, H, W = 16, 8, 64, 56, 56
HW = H * W  # 3136
NCORES = 8
BPC = B // NCORES  # batches per core = 2
CHUNK = 448  # 3136 = 7 * 448; one PSUM bank holds 448 fp32 comfortably
NCHUNK = HW // CHUNK

_PROGRAMS = {}


def _build_program(repeat=1, do_compute=True, do_out_dma=True, dual_ring=True):
    """repeat>1 wraps the whole body in a hardware loop; used only by the
    local benchmark harness to amortize dispatch overhead when timing.
    do_compute/do_out_dma isolate pipeline components for benchmarking."""
    import contextlib

    import concourse.bacc as bacc
    import concourse.mybir as mybir
    import concourse.tile as tile

    nc = bacc.Bacc("TRN2", debug=False)
    # am rows (partition = b*8 + c) with the selector blocks appended on the
    # free dim: one DMA covers both, so each matmul carries a single sem wait
    # (the Matmult instruction struct only has one sync-wait slot).
    amsel = nc.dram_tensor(
        "amsel", [BPC * C, HW + C * BPC * K], mybir.dt.float32, kind="ExternalInput"
    )
    first = nc.dram_tensor(
        "first", [BPC, K, HW], mybir.dt.float32, kind="ExternalInput"
    )
    out = nc.dram_tensor(
        "out", [BPC, C * K, HW], mybir.dt.float32, kind="ExternalOutput"
    )

    with tile.TileContext(nc) as tc:
        with (
            tc.tile_pool(name="ins", bufs=1) as ins_pool,
            tc.tile_pool(name="rep", bufs=8, space="PSUM") as psum_pool,
            tc.tile_pool(name="outs", bufs=3) as out_pool,
            tc.For_i(0, repeat, 1) if repeat > 1 else contextlib.nullcontext(),
        ):
            # first2[p] = first[p // 64, p % 64]  (both batches stacked)
            first2 = ins_pool.tile([BPC * K, HW], mybir.dt.float32)
            nc.sync.dma_start(
                out=first2[:], in_=first.ap().rearrange("b k f -> (b k) f")
            )
            # am2t[p, :HW] = am[p // 8, p % 8]  (partition = b*8 + c);
            # am2t[:, HW + c*128 : HW + (c+1)*128] = selector block for c.
            # With rhs = am2t[:, hw-chunk], sel_c.T @ rhs gives
            # rep[p, f] = am[p // 64, c, f]: block-broadcast of channel c of
            # each batch across that batch's 64 k-partitions. (PE requires rhs
            # base partition in {0, 32, 64}, so the selector — not a strided
            # rhs view — encodes the channel pick.)
            am2t = ins_pool.tile([BPC * C, HW + C * BPC * K], mybir.dt.float32)
            nc.sync.dma_start(out=am2t[:], in_=amsel.ap())

            out_ap = out.ap()
            for c in range(C):
                out_t = out_pool.tile([BPC * K, HW], mybir.dt.float32, tag="out")
                if not do_compute:
                    # bench-only: mark the tile written so sim allows the DMA
                    nc.vector.memset(out_t[:, 0:2], 0.0)
                if do_compute:
                    for j in range(NCHUNK):
                        f0 = j * CHUNK
                        rep = psum_pool.tile(
                            [BPC * K, CHUNK], mybir.dt.float32, tag="rep"
                        )
                        nc.tensor.matmul(
                            rep[:],
                            lhsT=am2t[
                                :, HW + c * BPC * K : HW + (c + 1) * BPC * K
                            ],
                            rhs=am2t[:, f0 : f0 + CHUNK],
                            start=True,
                            stop=True,
                        )
                        nc.vector.tensor_mul(
                            out_t[:, f0 : f0 + CHUNK],
                            first2[:, f0 : f0 + CHUNK],
                            rep[:],
                        )
                if do_out_dma:
                    # One DMA per batch ([64, HW] each, contiguous in DRAM).
                    # b=0 on the SP HWDGE ring, b=1 on the ACT ring — the two
                    # rings run concurrently so both partition halves are in
                    # flight and all 16 SBUF ports stay busy.
                    engs = (nc.sync, nc.scalar) if dual_ring else (nc.sync, nc.sync)
                    for b, eng in ((0, engs[0]), (1, engs[1])):
                        eng.dma_start(
                            out=out_ap[b, c * K : (c + 1) * K, :],
                            in_=out_t[b * K : (b + 1) * K, :],
                        )
    nc.compile()
    return nc


def _get_program(repeat=1, **variant):
    key = (repeat, tuple(sorted(variant.items())))
    if key not in _PROGRAMS:
        _PROGRAMS[key] = _build_program(repeat, **variant)
    return _PROGRAMS[key]


def _make_sel():
    # sel[b*C + c, c*128 + b*64 + k] = 1  (lhsT layout: [K=16, M=128] per c)
    sel = np.zeros((BPC * C, C * BPC * K), dtype=np.float32)
    for c in range(C):
        for b in range(BPC):
            sel[b * C + c, c * BPC * K + b * K : c * BPC * K + (b + 1) * K] = 1.0
    return sel


def _run(am_np, first_np, **spmd_kwargs):
    from concourse.bass_utils import run_bass_kernel_spmd

    nc = _get_program()
    sel = _make_sel()
    in_maps = []
    for i in range(NCORES):
        am_i = am_np[BPC * i : BPC * (i + 1)].reshape(BPC * C, HW)
        amsel = np.concatenate([am_i, sel], axis=1)
        in_maps.append(
            {
                "amsel": np.ascontiguousarray(amsel),
                "first": np.ascontiguousarray(first_np[BPC * i : BPC * (i + 1)]),
            }
        )
    return run_bass_kernel_spmd(nc, in_maps, core_ids=list(range(NCORES)), **spmd_kwargs)


def kernel(am_out, first_out):
    am_np = np.asarray(am_out, dtype=np.float32).reshape(B, C, HW)
    first_np = np.asarray(first_out, dtype=np.float32).reshape(B, K, HW)
    res = _run(am_np, first_np)
    out = np.concatenate([res.results[i]["out"] for i in range(NCORES)], axis=0)
    return out.reshape(B, C * K, H, W)


# revision 31
# speedup vs baseline: 6.3301x; 2.3921x over previous
"""ConnectionProductBlock on 8 TRN2 NeuronCores.

out[b, c*K + k, h, w] = am_out[b, c, h, w] * first_out[b, k, h, w]
  with B=16, C=8, K=64, H=W=56.

Strategy (data parallel over batch, 2 batches per core, no communication):
  - SBUF layout puts channels on partitions, hw (=3136) on the free dim so
    every DMA moves long contiguous runs (12.5KB per partition).
  - first_out for the core's 2 batches loads once as [128, 3136]
    (partition = b*64 + k).
  - am needs a partition-broadcast (am[b, c] replicated across the 64 k
    partitions of batch b). Compute engines have fixed lane<->partition
    wiring, so the replication is done on the idle TensorEngine: a K=2
    selector matmul sel.T @ am[{b0,b1}, c] writes rep[p, f] = am[p//64, c, f]
    into PSUM in 448-column chunks.
  - VectorEngine multiplies first * rep into an SBUF staging tile per c,
    which is DMAed out as one 1.6MB transfer.
HBM traffic per core is the 14.6MB minimum -> memory-roofline bound.
"""

import numpy as np

B, C, K, H, W = 16, 8, 64, 56, 56
HW = H * W  # 3136
NCORES = 8
BPC = B // NCORES  # batches per core = 2
CHUNK = 448  # 3136 = 7 * 448; one PSUM bank holds 448 fp32 comfortably
NCHUNK = HW // CHUNK

_PROGRAMS = {}


def _build_program(
    repeat=1,
    do_compute=True,
    do_out_dma=True,
    dual_ring=True,
    do_pe=True,
    do_mul=True,
    mul_src="psum",
):
    """repeat>1 wraps the whole body in a hardware loop; used only by the
    local benchmark harness to amortize dispatch overhead when timing.
    do_compute/do_out_dma isolate pipeline components for benchmarking."""
    import contextlib

    import concourse.bacc as bacc
    import concourse.mybir as mybir
    import concourse.tile as tile

    nc = bacc.Bacc("TRN2", debug=False)
    # am rows (partition = b*8 + c) with the selector blocks appended on the
    # free dim: one DMA covers both, so each matmul carries a single sem wait
    # (the Matmult instruction struct only has one sync-wait slot).
    amsel = nc.dram_tensor(
        "amsel", [BPC * C, HW + C * BPC * K], mybir.dt.float32, kind="ExternalInput"
    )
    first = nc.dram_tensor(
        "first", [BPC, K, HW], mybir.dt.float32, kind="ExternalInput"
    )
    out = nc.dram_tensor(
        "out", [BPC, C * K, HW], mybir.dt.float32, kind="ExternalOutput"
    )

    with tile.TileContext(nc) as tc:
        with (
            tc.tile_pool(name="ins", bufs=1) as ins_pool,
            tc.tile_pool(name="rep", bufs=8, space="PSUM") as psum_pool,
            tc.tile_pool(name="outs", bufs=3) as out_pool,
            tc.For_i(0, repeat, 1) if repeat > 1 else contextlib.nullcontext(),
        ):
            # first2[p] = first[p // 64, p % 64]  (both batches stacked)
            first2 = ins_pool.tile([BPC * K, HW], mybir.dt.float32)
            nc.sync.dma_start(
                out=first2[:], in_=first.ap().rearrange("b k f -> (b k) f")
            )
            # am2t[p, :HW] = am[p // 8, p % 8]  (partition = b*8 + c);
            # am2t[:, HW + c*128 : HW + (c+1)*128] = selector block for c.
            # With rhs = am2t[:, hw-chunk], sel_c.T @ rhs gives
            # rep[p, f] = am[p // 64, c, f]: block-broadcast of channel c of
            # each batch across that batch's 64 k-partitions. (PE requires rhs
            # base partition in {0, 32, 64}, so the selector — not a strided
            # rhs view — encodes the channel pick.)
            am2t = ins_pool.tile([BPC * C, HW + C * BPC * K], mybir.dt.float32)
            nc.sync.dma_start(out=am2t[:], in_=amsel.ap())

            out_ap = out.ap()
            for c in range(C):
                out_t = out_pool.tile([BPC * K, HW], mybir.dt.float32, tag="out")
                if not do_compute:
                    # bench-only: mark the tile written so sim allows the DMA
                    nc.vector.memset(out_t[:, 0:2], 0.0)
                if do_compute:
                    for j in range(NCHUNK):
                        f0 = j * CHUNK
                        rep = None
                        if do_pe:
                            rep = psum_pool.tile(
                                [BPC * K, CHUNK], mybir.dt.float32, tag="rep"
                            )
                            nc.tensor.matmul(
                                rep[:],
                                lhsT=am2t[
                                    :, HW + c * BPC * K : HW + (c + 1) * BPC * K
                                ],
                                rhs=am2t[:, f0 : f0 + CHUNK],
                                start=True,
                                stop=True,
                            )
                        if do_mul:
                            in1 = (
                                rep[:]
                                if (mul_src == "psum" and rep is not None)
                                else first2[:, f0 : f0 + CHUNK]
                            )
                            nc.vector.tensor_mul(
                                out_t[:, f0 : f0 + CHUNK],
                                first2[:, f0 : f0 + CHUNK],
                                in1,
                            )
                        elif do_pe:
                            pass
                    if not do_mul:
                        nc.vector.memset(out_t[:, 0:2], 0.0)
                if do_out_dma:
                    # One DMA per batch ([64, HW] each, contiguous in DRAM).
                    # b=0 on the SP HWDGE ring, b=1 on the ACT ring — the two
                    # rings run concurrently so both partition halves are in
                    # flight and all 16 SBUF ports stay busy.
                    engs = (nc.sync, nc.scalar) if dual_ring else (nc.sync, nc.sync)
                    for b, eng in ((0, engs[0]), (1, engs[1])):
                        eng.dma_start(
                            out=out_ap[b, c * K : (c + 1) * K, :],
                            in_=out_t[b * K : (b + 1) * K, :],
                        )
    nc.compile()
    return nc


def _get_program(repeat=1, **variant):
    key = (repeat, tuple(sorted(variant.items())))
    if key not in _PROGRAMS:
        _PROGRAMS[key] = _build_program(repeat, **variant)
    return _PROGRAMS[key]


def _make_sel():
    # sel[b*C + c, c*128 + b*64 + k] = 1  (lhsT layout: [K=16, M=128] per c)
    sel = np.zeros((BPC * C, C * BPC * K), dtype=np.float32)
    for c in range(C):
        for b in range(BPC):
            sel[b * C + c, c * BPC * K + b * K : c * BPC * K + (b + 1) * K] = 1.0
    return sel


def _run(am_np, first_np, **spmd_kwargs):
    from concourse.bass_utils import run_bass_kernel_spmd

    nc = _get_program()
    sel = _make_sel()
    in_maps = []
    for i in range(NCORES):
        am_i = am_np[BPC * i : BPC * (i + 1)].reshape(BPC * C, HW)
        amsel = np.concatenate([am_i, sel], axis=1)
        in_maps.append(
            {
                "amsel": np.ascontiguousarray(amsel),
                "first": np.ascontiguousarray(first_np[BPC * i : BPC * (i + 1)]),
            }
        )
    return run_bass_kernel_spmd(nc, in_maps, core_ids=list(range(NCORES)), **spmd_kwargs)


def kernel(am_out, first_out):
    am_np = np.asarray(am_out, dtype=np.float32).reshape(B, C, HW)
    first_np = np.asarray(first_out, dtype=np.float32).reshape(B, K, HW)
    res = _run(am_np, first_np)
    out = np.concatenate([res.results[i]["out"] for i in range(NCORES)], axis=0)
    return out.reshape(B, C * K, H, W)


# revision 34
# speedup vs baseline: 7.8264x; 1.2364x over previous
"""ConnectionProductBlock on 8 TRN2 NeuronCores.

out[b, c*K + k, h, w] = am_out[b, c, h, w] * first_out[b, k, h, w]
  with B=16, C=8, K=64, H=W=56.

Strategy (data parallel over batch, 2 batches per core, no communication):
  - SBUF layout puts channels on partitions, hw (=3136) on the free dim so
    every DMA moves long contiguous runs (12.5KB per partition).
  - first_out for the core's 2 batches loads once as [128, 3136]
    (partition = b*64 + k).
  - am needs a partition-broadcast (am[b, c] replicated across the 64 k
    partitions of batch b). Compute engines have fixed lane<->partition
    wiring, so the replication is done on the idle TensorEngine: a K=2
    selector matmul sel.T @ am[{b0,b1}, c] writes rep[p, f] = am[p//64, c, f]
    into PSUM in 448-column chunks.
  - VectorEngine multiplies first * rep into an SBUF staging tile per c,
    which is DMAed out as one 1.6MB transfer.
HBM traffic per core is the 14.6MB minimum -> memory-roofline bound.
"""

import numpy as np

B, C, K, H, W = 16, 8, 64, 56, 56
HW = H * W  # 3136
NCORES = 8
BPC = B // NCORES  # batches per core = 2
CHUNK = 448  # 3136 = 7 * 448; one PSUM bank holds 448 fp32 comfortably
NCHUNK = HW // CHUNK

_PROGRAMS = {}


def _build_program(
    repeat=1,
    do_compute=True,
    do_out_dma=True,
    dual_ring=True,
    do_pe=True,
    do_mul=True,
    mul_src="psum",
    pe_dtype="f32",
):
    """repeat>1 wraps the whole body in a hardware loop; used only by the
    local benchmark harness to amortize dispatch overhead when timing.
    do_compute/do_out_dma isolate pipeline components for benchmarking."""
    import contextlib

    import concourse.bacc as bacc
    import concourse.mybir as mybir
    import concourse.tile as tile

    nc = bacc.Bacc("TRN2", debug=False)
    # am rows (partition = b*8 + c) with the selector blocks appended on the
    # free dim: one DMA covers both, so each matmul carries a single sem wait
    # (the Matmult instruction struct only has one sync-wait slot).
    amsel = nc.dram_tensor(
        "amsel", [BPC * C, HW + C * BPC * K], mybir.dt.float32, kind="ExternalInput"
    )
    first = nc.dram_tensor(
        "first", [BPC, K, HW], mybir.dt.float32, kind="ExternalInput"
    )
    out = nc.dram_tensor(
        "out", [BPC, C * K, HW], mybir.dt.float32, kind="ExternalOutput"
    )

    with tile.TileContext(nc) as tc:
        with (
            tc.tile_pool(name="ins", bufs=1) as ins_pool,
            tc.tile_pool(name="rep", bufs=8, space="PSUM") as psum_pool,
            tc.tile_pool(name="outs", bufs=3) as out_pool,
            tc.For_i(0, repeat, 1) if repeat > 1 else contextlib.nullcontext(),
        ):
            # first2[p] = first[p // 64, p % 64]  (both batches stacked)
            first2 = ins_pool.tile([BPC * K, HW], mybir.dt.float32)
            nc.sync.dma_start(
                out=first2[:], in_=first.ap().rearrange("b k f -> (b k) f")
            )
            # am2t[p, :HW] = am[p // 8, p % 8]  (partition = b*8 + c);
            # am2t[:, HW + c*128 : HW + (c+1)*128] = selector block for c.
            # With rhs = am2t[:, hw-chunk], sel_c.T @ rhs gives
            # rep[p, f] = am[p // 64, c, f]: block-broadcast of channel c of
            # each batch across that batch's 64 k-partitions. (PE requires rhs
            # base partition in {0, 32, 64}, so the selector — not a strided
            # rhs view — encodes the channel pick.)
            am2t = ins_pool.tile([BPC * C, HW + C * BPC * K], mybir.dt.float32)
            nc.sync.dma_start(out=am2t[:], in_=amsel.ap())
            if pe_dtype == "bf16":
                # bench-only: casting copy of amsel for bf16 PE micro-bench
                am2t_bf = ins_pool.tile(
                    [BPC * C, HW + C * BPC * K], mybir.dt.bfloat16
                )
                nc.gpsimd.dma_start(out=am2t_bf[:], in_=amsel.ap())
                am2t_pe = am2t_bf
            else:
                am2t_pe = am2t

            out_ap = out.ap()
            for c in range(C):
                out_t = out_pool.tile([BPC * K, HW], mybir.dt.float32, tag="out")
                if not do_compute:
                    # bench-only: mark the tile written so sim allows the DMA
                    nc.vector.memset(out_t[:, 0:2], 0.0)
                if do_compute:
                    for j in range(NCHUNK):
                        f0 = j * CHUNK
                        rep = None
                        if do_pe:
                            rep = psum_pool.tile(
                                [BPC * K, CHUNK], mybir.dt.float32, tag="rep"
                            )
                            nc.tensor.matmul(
                                rep[:],
                                lhsT=am2t_pe[
                                    :, HW + c * BPC * K : HW + (c + 1) * BPC * K
                                ],
                                rhs=am2t_pe[:, f0 : f0 + CHUNK],
                                start=True,
                                stop=True,
                            )
                        if do_mul:
                            in1 = (
                                rep[:]
                                if (mul_src == "psum" and rep is not None)
                                else first2[:, f0 : f0 + CHUNK]
                            )
                            nc.vector.tensor_mul(
                                out_t[:, f0 : f0 + CHUNK],
                                first2[:, f0 : f0 + CHUNK],
                                in1,
                            )
                        elif do_pe:
                            pass
                    if not do_mul:
                        nc.vector.memset(out_t[:, 0:2], 0.0)
                if do_out_dma:
                    # One DMA per batch ([64, HW] each, contiguous in DRAM).
                    # b=0 on the SP HWDGE ring, b=1 on the ACT ring — the two
                    # rings run concurrently so both partition halves are in
                    # flight and all 16 SBUF ports stay busy.
                    engs = (nc.sync, nc.scalar) if dual_ring else (nc.sync, nc.sync)
                    for b, eng in ((0, engs[0]), (1, engs[1])):
                        eng.dma_start(
                            out=out_ap[b, c * K : (c + 1) * K, :],
                            in_=out_t[b * K : (b + 1) * K, :],
                        )
    nc.compile()
    return nc


def _get_program(repeat=1, **variant):
    key = (repeat, tuple(sorted(variant.items())))
    if key not in _PROGRAMS:
        _PROGRAMS[key] = _build_program(repeat, **variant)
    return _PROGRAMS[key]


def _make_sel():
    # sel[b*C + c, c*128 + b*64 + k] = 1  (lhsT layout: [K=16, M=128] per c)
    sel = np.zeros((BPC * C, C * BPC * K), dtype=np.float32)
    for c in range(C):
        for b in range(BPC):
            sel[b * C + c, c * BPC * K + b * K : c * BPC * K + (b + 1) * K] = 1.0
    return sel


def _run(am_np, first_np, **spmd_kwargs):
    from concourse.bass_utils import run_bass_kernel_spmd

    nc = _get_program()
    sel = _make_sel()
    in_maps = []
    for i in range(NCORES):
        am_i = am_np[BPC * i : BPC * (i + 1)].reshape(BPC * C, HW)
        amsel = np.concatenate([am_i, sel], axis=1)
        in_maps.append(
            {
                "amsel": np.ascontiguousarray(amsel),
                "first": np.ascontiguousarray(first_np[BPC * i : BPC * (i + 1)]),
            }
        )
    return run_bass_kernel_spmd(nc, in_maps, core_ids=list(range(NCORES)), **spmd_kwargs)


def kernel(am_out, first_out):
    am_np = np.asarray(am_out, dtype=np.float32).reshape(B, C, HW)
    first_np = np.asarray(first_out, dtype=np.float32).reshape(B, K, HW)
    res = _run(am_np, first_np)
    out = np.concatenate([res.results[i]["out"] for i in range(NCORES)], axis=0)
    return out.reshape(B, C * K, H, W)
